# revision 17
# baseline (speedup 1.0000x reference)
"""Kernel-score loss (RBF-MMD style) on 8 Trainium2 NeuronCores.

Math: let X = generated_samples.reshape(m, S*D), t = target_sample.reshape(-1)
and define X' = X - t (row-wise).  Then with G = X' @ X'.T (m x m):
  d2[i,j]  = ||X_i - X_j||^2  = ||X'_i - X'_j||^2 = G[i,i] + G[j,j] - 2 G[i,j]
  dt2[i]   = ||X_i - t||^2    = G[i,i]                (the t-shift absorbs it)
  cross    = (lambda/2) * (sum_{i!=j} exp(-g*d2)) / (m*(m-1))
  target   = mean_i exp(-g*dt2[i])
  score    = clip(cross - target, -10, 10)
so the single 64x64 Gram of the host-shifted samples carries the whole loss.

Sharding: the contraction axis (S*D = 524288) is split 8 ways.  Each core
receives its shard pre-packed k-major as A[c] of shape (128, 512, 64):
A[c][d, s, j] = X'[j, (c*512+s)*128 + d].  The device kernel streams its
4.19 MB shard once (memory-bound) and accumulates the partial Gram on the
PE; the host sums the 8 partial Grams and applies the 64x64 reduction.

v2 changes over the 34.3us baseline (trace-driven):
- The input stream (9.0->21.3us) is already at ~95% of the 358 GB/s
  per-NC HBM roofline; the real tail was the PE: matmuls ran COLD
  (HAM clock gate at 1.2 GHz until t=15.7us, ~53ns/chunk) and the PE
  ground on until 31.7us, 10.4us past the last input byte.
- PE warm-up: ~9 dummy N=512 matmuls issued at block start (t~7.6us)
  keep the PE busy until the first group's semaphore (~11.6us), so the
  HAM un-throttles (~3.4us of sustained activity) BEFORE the real
  stream begins and all 512 real matmuls run at the warm ~29ns rate.
- Group sizes [80, 48, 64*6]: the old uniform [64*8] left a ~0.45us
  stall at group 1 (PE finished group 0 at 13.5us, g1 sem ~13.95us).
  A bigger group 0 absorbs it (group-0's sem time is desc-gen-bound at
  128 descriptors ~2.24us regardless of chunk count, so growing it is
  free; the completion sems only pace the PE).
"""

import sys

import ml_dtypes
import numpy as np

if "/opt/trn_rl_repo" not in sys.path:
    sys.path.insert(0, "/opt/trn_rl_repo")

import concourse.bass as bass
import concourse.mybir as mybir
from concourse.bass_utils import run_bass_kernel_spmd

GAMMA = 1.0
LAMBDA = 0.5
CLAMP = (-10.0, 10.0)

M = 64          # samples
S = 4096        # time steps
D = 128         # feature dim
N_CORES = 8
S_SHARD = S // N_CORES          # 512 k-chunks per core

# DMA group sizes in k-chunks and their queue (0 = sync/qSP, 1 =
# scalar/qAct).  The HWDGE queues stream byte-paced at ~229 GB/s
# (~17.9ns/chunk) with qSP's first byte at ~9.0us and qAct's ~1.9us
# later, and each group's completion sem fires ~0.5us after its last
# byte.  Small leading groups on qSP pull the PE start to ~9.8us; the
# big tail groups ride the late queue, whose sems stay ahead of the
# 34ns/chunk warm PE.  (Schedule from a calibrated simulation; a
# uniform [64]*8 even/odd split measures ~1.9us slower.)
CHUNK_GROUPS = [24, 40, 64, 64, 64, 80, 88, 88]
GROUP_QUEUE = [0, 0, 0, 1, 0, 1, 0, 1]  # 0=sync/qSP, 1=scalar/qAct
assert sum(CHUNK_GROUPS) == S_SHARD
assert len(GROUP_QUEUE) == len(CHUNK_GROUPS)

# PE-side wait plan: which group sems to wait on before each group's
# matmuls.  Each HWDGE queue completes its groups in FIFO order, so one
# wait on a queue's LAST tail group implies all its earlier groups; in
# the tail the sems run ~3us ahead of the PE, so merging the last three
# groups' waits into one pair saves two ~250ns NX/pipeline bubbles.
WAIT_PLAN = [[0], [1], [2], [3], [4], [7, 6], [], []]

# PE warm-up: dummy matmuls issued before the first dma-sem wait.  Each
# N=512 fp8 matmul takes ~427ns cold (1.2 GHz); 5 of them span the
# 7.4->9.6us window so PE activity is continuous from 7.4us on and the
# HAM clock gate flips to 8/8 (2.4 GHz) at ~12.2us.
WARMUP_MMS = 8
WARMUP_N = 512

# 2x column tiling: run chunk pairs concurrently on the two 64-column
# halves of the PE array (tile_position (0,0)/(0,64)); each tile's
# output lands in its own PSUM partition half and the host sums halves.
COL_TILING = False

F32 = mybir.dt.float32
FP8 = mybir.dt.float8e4

_compiled = None


# Output rows: col tiling accumulates the Gram split across both PSUM
# partition halves (host sums them); otherwise rows 0..63 carry it all.
OUT_ROWS = 2 * M if COL_TILING else M


def _build_program():
    nc = bass.Bass()
    a = nc.declare_dram_parameter("a", [D, S_SHARD * M], FP8, isOutput=False)
    g = nc.declare_dram_parameter("g", [OUT_ROWS, M], F32, isOutput=True)

    import contextlib

    n_groups = len(CHUNK_GROUPS)
    with contextlib.ExitStack() as ctx:
        x_sb = ctx.enter_context(nc.sbuf_tensor([D, S_SHARD * M], FP8))
        warm_sb = ctx.enter_context(nc.sbuf_tensor([D, WARMUP_N], FP8))
        g_sb = ctx.enter_context(nc.sbuf_tensor([OUT_ROWS, M], F32))
        g_ps = ctx.enter_context(nc.psum_tensor([D, M], F32))
        warm_ps = ctx.enter_context(nc.psum_tensor([D, WARMUP_N], F32))
        dma_sems = [
            ctx.enter_context(nc.semaphore(f"dma_sem{i}")) for i in range(n_groups)
        ]
        out_sem = ctx.enter_context(nc.semaphore("out_sem"))
        pe_sem = ctx.enter_context(nc.semaphore("pe_sem"))
        dve_sem = ctx.enter_context(nc.semaphore("dve_sem"))
        block = ctx.enter_context(nc.Block())

        group_lo = np.cumsum([0] + CHUNK_GROUPS)

        def dma_group(eng, i):
            lo, hi = group_lo[i] * M, group_lo[i + 1] * M
            eng.dma_start(x_sb[:, lo:hi], a[:, lo:hi]).then_inc(dma_sems[i], 16)

        @block.gpsimd
        def _(gpsimd):
            # SWDGE: the gpsimd Q7 emits descriptors ~1us after its block
            # body starts (~7.3us), beating qSP's ~9.0us first byte, so the
            # PE's first group lands ~1.5us earlier.
            for i in range(n_groups):
                if GROUP_QUEUE[i] == 2:
                    dma_group(gpsimd, i)

        @block.sync
        def _(sync):
            for i in range(n_groups):
                if GROUP_QUEUE[i] == 0:
                    dma_group(sync, i)
            sync.wait_ge(dve_sem, 1)
            # Split output: each queue ships half the Gram as soon as its half
            # of the PSUM->SBUF copy lands (then_inc is engine-completion
            # ordered, same pattern as the verified single-DMA chain).  No
            # wait on the completion semaphores: the block-exit DRAIN flushes
            # the HWDGE queues and NRT fences DMA at NEFF end.
            sync.dma_start(
                g[: OUT_ROWS // 2, :], g_sb[: OUT_ROWS // 2, :]
            ).then_inc(out_sem, 16)

        @block.scalar
        def _(scalar):
            for i in range(n_groups):
                if GROUP_QUEUE[i] == 1:
                    dma_group(scalar, i)
            scalar.wait_ge(dve_sem, 2)
            scalar.dma_start(
                g[OUT_ROWS // 2 :, :], g_sb[OUT_ROWS // 2 :, :]
            ).then_inc(out_sem, 16)

        @block.vector
        def _(vector):
            vector.wait_ge(pe_sem, 1)
            nc.vector.tensor_copy(
                g_sb[: OUT_ROWS // 2, :], g_ps[: OUT_ROWS // 2, :]
            ).then_inc(dve_sem, 1)
            nc.vector.tensor_copy(
                g_sb[OUT_ROWS // 2 : OUT_ROWS, :], g_ps[OUT_ROWS // 2 : OUT_ROWS, :]
            ).then_inc(dve_sem, 1)

        @block.tensor
        def _(tensor):
            # HAM warm-up: garbage-in/garbage-out matmuls into a scratch PSUM
            # bank.  warm_sb is never written (fp8 garbage, possibly NaN);
            # warm_ps is never read.  These fill the PE queue before the
            # group-0 sem wait so the clock gate is at 8/8 when data lands.
            for _ in range(WARMUP_MMS):
                nc.tensor.matmul(
                    warm_ps[:, :],
                    warm_sb[:, :D],
                    warm_sb[:, :],
                    start=True,
                    stop=True,
                    skip_group_check=True,
                )
            if COL_TILING:
                # 2x column tiling: even chunks occupy PE array columns 0-63
                # (output PSUM partitions 0-63), odd chunks columns 64-127
                # (partitions 64-127); the two tiles' LDWEIGHTS+MATMULs run
                # concurrently on disjoint sub-arrays.  Only the very first
                # matmul uses start=True (clears the bank's has_written bits);
                # the other tile's first matmul overwrites where the bit is
                # unset, so both halves accumulate independently.  Host sums
                # G = P[0:64] + P[64:128].
                for i in range(n_groups):
                    tensor.wait_ge(dma_sems[i], 16)
                    for w in range(0, CHUNK_GROUPS[i], 2):
                        k = group_lo[i] + w
                        lo = k * M
                        last = k + 1 == S_SHARD - 1
                        nc.tensor.matmul(
                            g_ps[:M, :],
                            x_sb[:, lo : lo + M],
                            x_sb[:, lo : lo + M],
                            start=(k == 0),
                            stop=last,
                            skip_group_check=True,
                            tile_position=(0, 0),
                        )
                        inst = nc.tensor.matmul(
                            g_ps[M:, :],
                            x_sb[:, lo + M : lo + 2 * M],
                            x_sb[:, lo + M : lo + 2 * M],
                            start=False,
                            stop=last,
                            skip_group_check=True,
                            tile_position=(0, M),
                        )
                        if last:
                            inst.then_inc(pe_sem, 1)
            else:
                # Spill-FWL matmuls: the stationary AP spans 128 columns
                # (chunk k plus a spill into chunk k+1), triggering Fast
                # Weight Load; the junk only pollutes PSUM rows 64..127,
                # which are never read.  The last chunk of each group skips
                # the spill (its neighbour may not have landed yet) and runs
                # as a plain 64-col matmul.
                for i in range(n_groups):
                    for si in WAIT_PLAN[i]:
                        tensor.wait_ge(dma_sems[si], 16)
                    for w in range(CHUNK_GROUPS[i]):
                        k = group_lo[i] + w
                        lo = k * M
                        moving = x_sb[:, lo : lo + M]
                        if w != CHUNK_GROUPS[i] - 1:
                            stat = x_sb[:, lo : lo + 2 * M]
                            out = g_ps[:, :]
                        else:
                            stat = moving
                            out = g_ps[:M, :]
                        inst = nc.tensor.matmul(
                            out,
                            stat,
                            moving,
                            start=(k == 0),
                            stop=(k == S_SHARD - 1),
                            skip_group_check=True,
                        )
                        if k == S_SHARD - 1:
                            inst.then_inc(pe_sem, 1)

    return nc


def _get_program():
    global _compiled
    if _compiled is None:
        _compiled = _build_program()
    return _compiled


def _shard_inputs(generated_samples, target_sample):
    # A[c][d, s, j] = (X - t)[j, (c*512+s)*128 + d]
    x = np.asarray(generated_samples, dtype=np.float32)
    t = np.asarray(target_sample, dtype=np.float32)
    xs = x - t[None, :, :]                        # (M, S, D)
    # (M, S, D) -> view (M, N_CORES, S_SHARD, D) -> (N_CORES, D, S_SHARD, M)
    a = xs.reshape(M, N_CORES, S_SHARD, D).transpose(1, 3, 2, 0)
    a8 = np.ascontiguousarray(a).astype(ml_dtypes.float8_e4m3)
    return [{"a": a8[c].reshape(D, S_SHARD * M)} for c in range(N_CORES)]


def _finalize(G):
    # G: (64, 64) float64 summed Gram of X' = X - t
    sq = np.diag(G)
    d2 = np.maximum(sq[:, None] + sq[None, :] - 2.0 * G, 0.0)
    K = np.exp(-GAMMA * d2)
    cross_sum = np.sum(K) - np.trace(K)
    cross_term = (LAMBDA / 2.0) * cross_sum / (M * (M - 1))
    target_term = np.mean(np.exp(-GAMMA * sq))
    score = np.clip(cross_term - target_term, CLAMP[0], CLAMP[1])
    return np.float32(score)


def _run(generated_samples, target_sample, time_points=None, trace=False):
    nc = _get_program()
    in_maps = _shard_inputs(generated_samples, target_sample)
    res = run_bass_kernel_spmd(nc, in_maps, list(range(N_CORES)), trace=trace)
    G = np.zeros((M, M), dtype=np.float64)
    for r in res.results:
        gg = np.asarray(r["g"], dtype=np.float64)
        if gg.shape[0] == 2 * M:  # col-tiled: sum the partition halves
            gg = gg[:M, :] + gg[M:, :]
        G += gg
    return _finalize(G), res


def kernel(generated_samples, target_sample, time_points=None):
    out, _ = _run(generated_samples, target_sample, time_points)
    return out


# revision 18
# speedup vs baseline: 1.0330x; 1.0330x over previous
"""Kernel-score loss (RBF-MMD style) on 8 Trainium2 NeuronCores.

Math: let X = generated_samples.reshape(m, S*D), t = target_sample.reshape(-1)
and define X' = X - t (row-wise).  Then with G = X' @ X'.T (m x m):
  d2[i,j]  = ||X_i - X_j||^2  = ||X'_i - X'_j||^2 = G[i,i] + G[j,j] - 2 G[i,j]
  dt2[i]   = ||X_i - t||^2    = G[i,i]                (the t-shift absorbs it)
  cross    = (lambda/2) * (sum_{i!=j} exp(-g*d2)) / (m*(m-1))
  target   = mean_i exp(-g*dt2[i])
  score    = clip(cross - target, -10, 10)
so the single 64x64 Gram of the host-shifted samples carries the whole loss.

Sharding: the contraction axis (S*D = 524288) is split 8 ways.  Each core
receives its shard pre-packed k-major as A[c] of shape (128, 512, 64):
A[c][d, s, j] = X'[j, (c*512+s)*128 + d].  The device kernel streams its
4.19 MB shard once (memory-bound) and accumulates the partial Gram on the
PE; the host sums the 8 partial Grams and applies the 64x64 reduction.

v2 changes over the 34.3us baseline (trace-driven):
- The input stream (9.0->21.3us) is already at ~95% of the 358 GB/s
  per-NC HBM roofline; the real tail was the PE: matmuls ran COLD
  (HAM clock gate at 1.2 GHz until t=15.7us, ~53ns/chunk) and the PE
  ground on until 31.7us, 10.4us past the last input byte.
- PE warm-up: ~9 dummy N=512 matmuls issued at block start (t~7.6us)
  keep the PE busy until the first group's semaphore (~11.6us), so the
  HAM un-throttles (~3.4us of sustained activity) BEFORE the real
  stream begins and all 512 real matmuls run at the warm ~29ns rate.
- Group sizes [80, 48, 64*6]: the old uniform [64*8] left a ~0.45us
  stall at group 1 (PE finished group 0 at 13.5us, g1 sem ~13.95us).
  A bigger group 0 absorbs it (group-0's sem time is desc-gen-bound at
  128 descriptors ~2.24us regardless of chunk count, so growing it is
  free; the completion sems only pace the PE).
"""

import sys

import ml_dtypes
import numpy as np

if "/opt/trn_rl_repo" not in sys.path:
    sys.path.insert(0, "/opt/trn_rl_repo")

import concourse.bass as bass
import concourse.mybir as mybir
from concourse.bass_utils import run_bass_kernel_spmd

GAMMA = 1.0
LAMBDA = 0.5
CLAMP = (-10.0, 10.0)

M = 64          # samples
S = 4096        # time steps
D = 128         # feature dim
N_CORES = 8
S_SHARD = S // N_CORES          # 512 k-chunks per core

# DMA group sizes in k-chunks and their queue (0 = sync/qSP, 1 =
# scalar/qAct).  The HWDGE queues stream byte-paced at ~229 GB/s
# (~17.9ns/chunk) with qSP's first byte at ~9.0us and qAct's ~1.9us
# later, and each group's completion sem fires ~0.5us after its last
# byte.  Small leading groups on qSP pull the PE start to ~9.8us; the
# big tail groups ride the late queue, whose sems stay ahead of the
# 34ns/chunk warm PE.  (Schedule from a calibrated simulation; a
# uniform [64]*8 even/odd split measures ~1.9us slower.)
CHUNK_GROUPS = [32, 48, 96, 112, 64, 80, 80]
GROUP_QUEUE = [0, 0, 1, 1, 0, 1, 1]  # 0=sync/qSP, 1=scalar/qAct
assert sum(CHUNK_GROUPS) == S_SHARD
assert len(GROUP_QUEUE) == len(CHUNK_GROUPS)

# PE-side wait plan: which group sems to wait on before each group's
# matmuls.  Each HWDGE queue completes its groups in FIFO order, so one
# wait on a queue's LAST tail group implies all its earlier groups; in
# the tail the sems run ~3us ahead of the PE, so merging the last three
# groups' waits into one pair saves two ~250ns NX/pipeline bubbles.
def _wait_plan():
    n = len(CHUNK_GROUPS)
    plan = [[i] for i in range(n)]
    tail = list(range(n - 3, n))
    merged = []
    for q in (0, 1):
        qtail = [i for i in tail if GROUP_QUEUE[i] == q]
        if qtail:
            merged.append(max(qtail))
    plan[n - 3] = sorted(merged, reverse=True)
    plan[n - 2] = []
    plan[n - 1] = []
    return plan


WAIT_PLAN = _wait_plan()

# PE warm-up: dummy matmuls issued before the first dma-sem wait.  Each
# N=512 fp8 matmul takes ~427ns cold (1.2 GHz); 5 of them span the
# 7.4->9.6us window so PE activity is continuous from 7.4us on and the
# HAM clock gate flips to 8/8 (2.4 GHz) at ~12.2us.
WARMUP_MMS = 7
WARMUP_N = 512

# 2x column tiling: run chunk pairs concurrently on the two 64-column
# halves of the PE array (tile_position (0,0)/(0,64)); each tile's
# output lands in its own PSUM partition half and the host sums halves.
COL_TILING = False

F32 = mybir.dt.float32
FP8 = mybir.dt.float8e4

_compiled = None


# Output rows: col tiling accumulates the Gram split across both PSUM
# partition halves (host sums them); otherwise rows 0..63 carry it all.
OUT_ROWS = 2 * M if COL_TILING else M


def _build_program():
    nc = bass.Bass()
    a = nc.declare_dram_parameter("a", [D, S_SHARD * M], FP8, isOutput=False)
    g = nc.declare_dram_parameter("g", [OUT_ROWS, M], F32, isOutput=True)

    import contextlib

    n_groups = len(CHUNK_GROUPS)
    with contextlib.ExitStack() as ctx:
        x_sb = ctx.enter_context(nc.sbuf_tensor([D, S_SHARD * M], FP8))
        warm_sb = ctx.enter_context(nc.sbuf_tensor([D, WARMUP_N], FP8))
        g_sb = ctx.enter_context(nc.sbuf_tensor([OUT_ROWS, M], F32))
        g_ps = ctx.enter_context(nc.psum_tensor([D, M], F32))
        warm_ps = ctx.enter_context(nc.psum_tensor([D, WARMUP_N], F32))
        dma_sems = [
            ctx.enter_context(nc.semaphore(f"dma_sem{i}")) for i in range(n_groups)
        ]
        out_sem = ctx.enter_context(nc.semaphore("out_sem"))
        pe_sem = ctx.enter_context(nc.semaphore("pe_sem"))
        dve_sem = ctx.enter_context(nc.semaphore("dve_sem"))
        block = ctx.enter_context(nc.Block())

        group_lo = np.cumsum([0] + CHUNK_GROUPS)

        def dma_group(eng, i):
            lo, hi = group_lo[i] * M, group_lo[i + 1] * M
            eng.dma_start(x_sb[:, lo:hi], a[:, lo:hi]).then_inc(dma_sems[i], 16)

        @block.gpsimd
        def _(gpsimd):
            # SWDGE: the gpsimd Q7 emits descriptors ~1us after its block
            # body starts (~7.3us), beating qSP's ~9.0us first byte, so the
            # PE's first group lands ~1.5us earlier.
            for i in range(n_groups):
                if GROUP_QUEUE[i] == 2:
                    dma_group(gpsimd, i)

        @block.sync
        def _(sync):
            for i in range(n_groups):
                if GROUP_QUEUE[i] == 0:
                    dma_group(sync, i)
            sync.wait_ge(dve_sem, 1)
            # Split output: each queue ships half the Gram as soon as its half
            # of the PSUM->SBUF copy lands (then_inc is engine-completion
            # ordered, same pattern as the verified single-DMA chain).  No
            # wait on the completion semaphores: the block-exit DRAIN flushes
            # the HWDGE queues and NRT fences DMA at NEFF end.
            sync.dma_start(
                g[: OUT_ROWS // 2, :], g_sb[: OUT_ROWS // 2, :]
            ).then_inc(out_sem, 16)

        @block.scalar
        def _(scalar):
            for i in range(n_groups):
                if GROUP_QUEUE[i] == 1:
                    dma_group(scalar, i)
            scalar.wait_ge(dve_sem, 2)
            scalar.dma_start(
                g[OUT_ROWS // 2 :, :], g_sb[OUT_ROWS // 2 :, :]
            ).then_inc(out_sem, 16)

        @block.vector
        def _(vector):
            vector.wait_ge(pe_sem, 1)
            nc.vector.tensor_copy(
                g_sb[: OUT_ROWS // 2, :], g_ps[: OUT_ROWS // 2, :]
            ).then_inc(dve_sem, 1)
            nc.vector.tensor_copy(
                g_sb[OUT_ROWS // 2 : OUT_ROWS, :], g_ps[OUT_ROWS // 2 : OUT_ROWS, :]
            ).then_inc(dve_sem, 1)

        @block.tensor
        def _(tensor):
            # HAM warm-up: garbage-in/garbage-out matmuls into a scratch PSUM
            # bank.  warm_sb is never written (fp8 garbage, possibly NaN);
            # warm_ps is never read.  These fill the PE queue before the
            # group-0 sem wait so the clock gate is at 8/8 when data lands.
            for _ in range(WARMUP_MMS):
                nc.tensor.matmul(
                    warm_ps[:, :],
                    warm_sb[:, :D],
                    warm_sb[:, :],
                    start=True,
                    stop=True,
                    skip_group_check=True,
                )
            if COL_TILING:
                # 2x column tiling: even chunks occupy PE array columns 0-63
                # (output PSUM partitions 0-63), odd chunks columns 64-127
                # (partitions 64-127); the two tiles' LDWEIGHTS+MATMULs run
                # concurrently on disjoint sub-arrays.  Only the very first
                # matmul uses start=True (clears the bank's has_written bits);
                # the other tile's first matmul overwrites where the bit is
                # unset, so both halves accumulate independently.  Host sums
                # G = P[0:64] + P[64:128].
                for i in range(n_groups):
                    tensor.wait_ge(dma_sems[i], 16)
                    for w in range(0, CHUNK_GROUPS[i], 2):
                        k = group_lo[i] + w
                        lo = k * M
                        last = k + 1 == S_SHARD - 1
                        nc.tensor.matmul(
                            g_ps[:M, :],
                            x_sb[:, lo : lo + M],
                            x_sb[:, lo : lo + M],
                            start=(k == 0),
                            stop=last,
                            skip_group_check=True,
                            tile_position=(0, 0),
                        )
                        inst = nc.tensor.matmul(
                            g_ps[M:, :],
                            x_sb[:, lo + M : lo + 2 * M],
                            x_sb[:, lo + M : lo + 2 * M],
                            start=False,
                            stop=last,
                            skip_group_check=True,
                            tile_position=(0, M),
                        )
                        if last:
                            inst.then_inc(pe_sem, 1)
            else:
                # Spill-FWL matmuls: the stationary AP spans 128 columns
                # (chunk k plus a spill into chunk k+1), triggering Fast
                # Weight Load; the junk only pollutes PSUM rows 64..127,
                # which are never read.  The last chunk of each group skips
                # the spill (its neighbour may not have landed yet) and runs
                # as a plain 64-col matmul.
                for i in range(n_groups):
                    for si in WAIT_PLAN[i]:
                        tensor.wait_ge(dma_sems[si], 16)
                    for w in range(CHUNK_GROUPS[i]):
                        k = group_lo[i] + w
                        lo = k * M
                        moving = x_sb[:, lo : lo + M]
                        if w != CHUNK_GROUPS[i] - 1:
                            stat = x_sb[:, lo : lo + 2 * M]
                            out = g_ps[:, :]
                        else:
                            stat = moving
                            out = g_ps[:M, :]
                        inst = nc.tensor.matmul(
                            out,
                            stat,
                            moving,
                            start=(k == 0),
                            stop=(k == S_SHARD - 1),
                            skip_group_check=True,
                        )
                        if k == S_SHARD - 1:
                            inst.then_inc(pe_sem, 1)

    return nc


def _get_program():
    global _compiled
    if _compiled is None:
        _compiled = _build_program()
    return _compiled


def _shard_inputs(generated_samples, target_sample):
    # A[c][d, s, j] = (X - t)[j, (c*512+s)*128 + d]
    x = np.asarray(generated_samples, dtype=np.float32)
    t = np.asarray(target_sample, dtype=np.float32)
    xs = x - t[None, :, :]                        # (M, S, D)
    # (M, S, D) -> view (M, N_CORES, S_SHARD, D) -> (N_CORES, D, S_SHARD, M)
    a = xs.reshape(M, N_CORES, S_SHARD, D).transpose(1, 3, 2, 0)
    a8 = np.ascontiguousarray(a).astype(ml_dtypes.float8_e4m3)
    return [{"a": a8[c].reshape(D, S_SHARD * M)} for c in range(N_CORES)]


def _finalize(G):
    # G: (64, 64) float64 summed Gram of X' = X - t
    sq = np.diag(G)
    d2 = np.maximum(sq[:, None] + sq[None, :] - 2.0 * G, 0.0)
    K = np.exp(-GAMMA * d2)
    cross_sum = np.sum(K) - np.trace(K)
    cross_term = (LAMBDA / 2.0) * cross_sum / (M * (M - 1))
    target_term = np.mean(np.exp(-GAMMA * sq))
    score = np.clip(cross_term - target_term, CLAMP[0], CLAMP[1])
    return np.float32(score)


def _run(generated_samples, target_sample, time_points=None, trace=False):
    nc = _get_program()
    in_maps = _shard_inputs(generated_samples, target_sample)
    res = run_bass_kernel_spmd(nc, in_maps, list(range(N_CORES)), trace=trace)
    G = np.zeros((M, M), dtype=np.float64)
    for r in res.results:
        gg = np.asarray(r["g"], dtype=np.float64)
        if gg.shape[0] == 2 * M:  # col-tiled: sum the partition halves
            gg = gg[:M, :] + gg[M:, :]
        G += gg
    return _finalize(G), res


def kernel(generated_samples, target_sample, time_points=None):
    out, _ = _run(generated_samples, target_sample, time_points)
    return out


# revision 19
# speedup vs baseline: 1.1151x; 1.0794x over previous
"""Kernel-score loss (RBF-MMD style) on 8 Trainium2 NeuronCores.

Math: let X = generated_samples.reshape(m, S*D), t = target_sample.reshape(-1)
and define X' = X - t (row-wise).  Then with G = X' @ X'.T (m x m):
  d2[i,j]  = ||X_i - X_j||^2  = ||X'_i - X'_j||^2 = G[i,i] + G[j,j] - 2 G[i,j]
  dt2[i]   = ||X_i - t||^2    = G[i,i]                (the t-shift absorbs it)
  cross    = (lambda/2) * (sum_{i!=j} exp(-g*d2)) / (m*(m-1))
  target   = mean_i exp(-g*dt2[i])
  score    = clip(cross - target, -10, 10)
so the single 64x64 Gram of the host-shifted samples carries the whole loss.

Sharding: the contraction axis (S*D = 524288) is split 8 ways.  Each core
receives its shard pre-packed k-major as A[c] of shape (128, 512, 64):
A[c][d, s, j] = X'[j, (c*512+s)*128 + d].  The device kernel streams its
4.19 MB shard once (memory-bound) and accumulates the partial Gram on the
PE; the host sums the 8 partial Grams and applies the 64x64 reduction.

v2 changes over the 34.3us baseline (trace-driven):
- The input stream (9.0->21.3us) is already at ~95% of the 358 GB/s
  per-NC HBM roofline; the real tail was the PE: matmuls ran COLD
  (HAM clock gate at 1.2 GHz until t=15.7us, ~53ns/chunk) and the PE
  ground on until 31.7us, 10.4us past the last input byte.
- PE warm-up: ~9 dummy N=512 matmuls issued at block start (t~7.6us)
  keep the PE busy until the first group's semaphore (~11.6us), so the
  HAM un-throttles (~3.4us of sustained activity) BEFORE the real
  stream begins and all 512 real matmuls run at the warm ~29ns rate.
- Group sizes [80, 48, 64*6]: the old uniform [64*8] left a ~0.45us
  stall at group 1 (PE finished group 0 at 13.5us, g1 sem ~13.95us).
  A bigger group 0 absorbs it (group-0's sem time is desc-gen-bound at
  128 descriptors ~2.24us regardless of chunk count, so growing it is
  free; the completion sems only pace the PE).
"""

import sys

import ml_dtypes
import numpy as np

if "/opt/trn_rl_repo" not in sys.path:
    sys.path.insert(0, "/opt/trn_rl_repo")

import concourse.bass as bass
import concourse.mybir as mybir
from concourse.bass_utils import run_bass_kernel_spmd

GAMMA = 1.0
LAMBDA = 0.5
CLAMP = (-10.0, 10.0)

M = 64          # samples
S = 4096        # time steps
D = 128         # feature dim
N_CORES = 8
S_SHARD = S // N_CORES          # 512 k-chunks per core

# DMA group sizes in k-chunks and their queue (0 = sync/qSP, 1 =
# scalar/qAct).  The HWDGE queues stream byte-paced at ~229 GB/s
# (~17.9ns/chunk) with qSP's first byte at ~9.0us and qAct's ~1.9us
# later, and each group's completion sem fires ~0.5us after its last
# byte.  Small leading groups on qSP pull the PE start to ~9.8us; the
# big tail groups ride the late queue, whose sems stay ahead of the
# 34ns/chunk warm PE.  (Schedule from a calibrated simulation; a
# uniform [64]*8 even/odd split measures ~1.9us slower.)
CHUNK_GROUPS = [32, 64, 64, 64, 96, 64, 64, 64]
GROUP_QUEUE = [0, 1, 0, 1, 0, 1, 0, 1]  # 0=sync/qSP, 1=scalar/qAct
assert sum(CHUNK_GROUPS) == S_SHARD
assert len(GROUP_QUEUE) == len(CHUNK_GROUPS)

# PE-side wait plan: which group sems to wait on before each group's
# matmuls.  Each HWDGE queue completes its groups in FIFO order, so one
# wait on a queue's LAST tail group implies all its earlier groups; in
# the tail the sems run ~3us ahead of the PE, so merging the last three
# groups' waits into one pair saves two ~250ns NX/pipeline bubbles.
def _wait_plan():
    n = len(CHUNK_GROUPS)
    plan = [[i] for i in range(n)]
    tail = list(range(n - 3, n))
    merged = []
    for q in (0, 1):
        qtail = [i for i in tail if GROUP_QUEUE[i] == q]
        if qtail:
            merged.append(max(qtail))
    plan[n - 3] = sorted(merged, reverse=True)
    plan[n - 2] = []
    plan[n - 1] = []
    return plan


WAIT_PLAN = _wait_plan()

# PE warm-up: dummy matmuls issued before the first dma-sem wait.  Each
# N=512 fp8 matmul takes ~427ns cold (1.2 GHz); 5 of them span the
# 7.4->9.6us window so PE activity is continuous from 7.4us on and the
# HAM clock gate flips to 8/8 (2.4 GHz) at ~12.2us.
WARMUP_MMS = 9
WARMUP_N = 512

# 2x column tiling: run chunk pairs concurrently on the two 64-column
# halves of the PE array (tile_position (0,0)/(0,64)); each tile's
# output lands in its own PSUM partition half and the host sums halves.
COL_TILING = False

F32 = mybir.dt.float32
FP8 = mybir.dt.float8e4

_compiled = None


# Output rows: col tiling accumulates the Gram split across both PSUM
# partition halves (host sums them); otherwise rows 0..63 carry it all.
OUT_ROWS = 2 * M if COL_TILING else M


def _build_program():
    nc = bass.Bass()
    a = nc.declare_dram_parameter("a", [D, S_SHARD * M], FP8, isOutput=False)
    g = nc.declare_dram_parameter("g", [OUT_ROWS, M], F32, isOutput=True)

    import contextlib

    n_groups = len(CHUNK_GROUPS)
    with contextlib.ExitStack() as ctx:
        x_sb = ctx.enter_context(nc.sbuf_tensor([D, S_SHARD * M], FP8))
        warm_sb = ctx.enter_context(nc.sbuf_tensor([D, WARMUP_N], FP8))
        g_sb = ctx.enter_context(nc.sbuf_tensor([OUT_ROWS, M], F32))
        g_ps = ctx.enter_context(nc.psum_tensor([D, M], F32))
        warm_ps = ctx.enter_context(nc.psum_tensor([D, WARMUP_N], F32))
        dma_sems = [
            ctx.enter_context(nc.semaphore(f"dma_sem{i}")) for i in range(n_groups)
        ]
        out_sem = ctx.enter_context(nc.semaphore("out_sem"))
        pe_sem = ctx.enter_context(nc.semaphore("pe_sem"))
        dve_sem = ctx.enter_context(nc.semaphore("dve_sem"))
        block = ctx.enter_context(nc.Block())

        group_lo = np.cumsum([0] + CHUNK_GROUPS)

        def dma_group(eng, i):
            lo, hi = group_lo[i] * M, group_lo[i + 1] * M
            eng.dma_start(x_sb[:, lo:hi], a[:, lo:hi]).then_inc(dma_sems[i], 16)

        @block.gpsimd
        def _(gpsimd):
            # SWDGE: the gpsimd Q7 emits descriptors ~1us after its block
            # body starts (~7.3us), beating qSP's ~9.0us first byte, so the
            # PE's first group lands ~1.5us earlier.
            for i in range(n_groups):
                if GROUP_QUEUE[i] == 2:
                    dma_group(gpsimd, i)

        @block.sync
        def _(sync):
            for i in range(n_groups):
                if GROUP_QUEUE[i] == 0:
                    dma_group(sync, i)
            sync.wait_ge(dve_sem, 1)
            # Split output: each queue ships half the Gram as soon as its half
            # of the PSUM->SBUF copy lands (then_inc is engine-completion
            # ordered, same pattern as the verified single-DMA chain).  No
            # wait on the completion semaphores: the block-exit DRAIN flushes
            # the HWDGE queues and NRT fences DMA at NEFF end.
            sync.dma_start(
                g[: OUT_ROWS // 2, :], g_sb[: OUT_ROWS // 2, :]
            ).then_inc(out_sem, 16)

        @block.scalar
        def _(scalar):
            for i in range(n_groups):
                if GROUP_QUEUE[i] == 1:
                    dma_group(scalar, i)
            scalar.wait_ge(dve_sem, 2)
            scalar.dma_start(
                g[OUT_ROWS // 2 :, :], g_sb[OUT_ROWS // 2 :, :]
            ).then_inc(out_sem, 16)

        @block.vector
        def _(vector):
            vector.wait_ge(pe_sem, 1)
            nc.vector.tensor_copy(
                g_sb[: OUT_ROWS // 2, :], g_ps[: OUT_ROWS // 2, :]
            ).then_inc(dve_sem, 1)
            nc.vector.tensor_copy(
                g_sb[OUT_ROWS // 2 : OUT_ROWS, :], g_ps[OUT_ROWS // 2 : OUT_ROWS, :]
            ).then_inc(dve_sem, 1)

        @block.tensor
        def _(tensor):
            # HAM warm-up: garbage-in/garbage-out matmuls into a scratch PSUM
            # bank.  warm_sb is never written (fp8 garbage, possibly NaN);
            # warm_ps is never read.  These fill the PE queue before the
            # group-0 sem wait so the clock gate is at 8/8 when data lands.
            for _ in range(WARMUP_MMS):
                nc.tensor.matmul(
                    warm_ps[:, :],
                    warm_sb[:, :D],
                    warm_sb[:, :],
                    start=True,
                    stop=True,
                    skip_group_check=True,
                )
            if COL_TILING:
                # 2x column tiling: even chunks occupy PE array columns 0-63
                # (output PSUM partitions 0-63), odd chunks columns 64-127
                # (partitions 64-127); the two tiles' LDWEIGHTS+MATMULs run
                # concurrently on disjoint sub-arrays.  Only the very first
                # matmul uses start=True (clears the bank's has_written bits);
                # the other tile's first matmul overwrites where the bit is
                # unset, so both halves accumulate independently.  Host sums
                # G = P[0:64] + P[64:128].
                for i in range(n_groups):
                    tensor.wait_ge(dma_sems[i], 16)
                    for w in range(0, CHUNK_GROUPS[i], 2):
                        k = group_lo[i] + w
                        lo = k * M
                        last = k + 1 == S_SHARD - 1
                        nc.tensor.matmul(
                            g_ps[:M, :],
                            x_sb[:, lo : lo + M],
                            x_sb[:, lo : lo + M],
                            start=(k == 0),
                            stop=last,
                            skip_group_check=True,
                            tile_position=(0, 0),
                        )
                        inst = nc.tensor.matmul(
                            g_ps[M:, :],
                            x_sb[:, lo + M : lo + 2 * M],
                            x_sb[:, lo + M : lo + 2 * M],
                            start=False,
                            stop=last,
                            skip_group_check=True,
                            tile_position=(0, M),
                        )
                        if last:
                            inst.then_inc(pe_sem, 1)
            else:
                # Spill-FWL matmuls: the stationary AP spans 128 columns
                # (chunk k plus a spill into chunk k+1), triggering Fast
                # Weight Load; the junk only pollutes PSUM rows 64..127,
                # which are never read.  The last chunk of each group skips
                # the spill (its neighbour may not have landed yet) and runs
                # as a plain 64-col matmul.
                for i in range(n_groups):
                    for si in WAIT_PLAN[i]:
                        tensor.wait_ge(dma_sems[si], 16)
                    for w in range(CHUNK_GROUPS[i]):
                        k = group_lo[i] + w
                        lo = k * M
                        moving = x_sb[:, lo : lo + M]
                        if w != CHUNK_GROUPS[i] - 1:
                            stat = x_sb[:, lo : lo + 2 * M]
                            out = g_ps[:, :]
                        else:
                            stat = moving
                            out = g_ps[:M, :]
                        inst = nc.tensor.matmul(
                            out,
                            stat,
                            moving,
                            start=(k == 0),
                            stop=(k == S_SHARD - 1),
                            skip_group_check=True,
                        )
                        if k == S_SHARD - 1:
                            inst.then_inc(pe_sem, 1)

    return nc


def _get_program():
    global _compiled
    if _compiled is None:
        _compiled = _build_program()
    return _compiled


def _shard_inputs(generated_samples, target_sample):
    # A[c][d, s, j] = (X - t)[j, (c*512+s)*128 + d]
    x = np.asarray(generated_samples, dtype=np.float32)
    t = np.asarray(target_sample, dtype=np.float32)
    xs = x - t[None, :, :]                        # (M, S, D)
    # (M, S, D) -> view (M, N_CORES, S_SHARD, D) -> (N_CORES, D, S_SHARD, M)
    a = xs.reshape(M, N_CORES, S_SHARD, D).transpose(1, 3, 2, 0)
    a8 = np.ascontiguousarray(a).astype(ml_dtypes.float8_e4m3)
    return [{"a": a8[c].reshape(D, S_SHARD * M)} for c in range(N_CORES)]


def _finalize(G):
    # G: (64, 64) float64 summed Gram of X' = X - t
    sq = np.diag(G)
    d2 = np.maximum(sq[:, None] + sq[None, :] - 2.0 * G, 0.0)
    K = np.exp(-GAMMA * d2)
    cross_sum = np.sum(K) - np.trace(K)
    cross_term = (LAMBDA / 2.0) * cross_sum / (M * (M - 1))
    target_term = np.mean(np.exp(-GAMMA * sq))
    score = np.clip(cross_term - target_term, CLAMP[0], CLAMP[1])
    return np.float32(score)


def _run(generated_samples, target_sample, time_points=None, trace=False):
    nc = _get_program()
    in_maps = _shard_inputs(generated_samples, target_sample)
    res = run_bass_kernel_spmd(nc, in_maps, list(range(N_CORES)), trace=trace)
    G = np.zeros((M, M), dtype=np.float64)
    for r in res.results:
        gg = np.asarray(r["g"], dtype=np.float64)
        if gg.shape[0] == 2 * M:  # col-tiled: sum the partition halves
            gg = gg[:M, :] + gg[M:, :]
        G += gg
    return _finalize(G), res


def kernel(generated_samples, target_sample, time_points=None):
    out, _ = _run(generated_samples, target_sample, time_points)
    return out
